# revision 38
# baseline (speedup 1.0000x reference)
"""GQA attention kernel for 8 TRN2 NeuronCores (Bass/Tile) — v4.

Problem: h[2,2048,1024] -> out[2,2048,1024]
  q = h @ wq_w.T + wq_b   (16 heads x 64)
  k/v = h @ w{k,v}_w.T + b (4 KV groups x 64, each serves 4 consecutive heads)
  out = softmax(q k^T / 8) v

Sharding: 8 cores = 2 batches x 4 KV groups (4 query heads each, one shared
K/V group per core). Fully independent, no collectives.

v4 structure (baselines: v2 172.4us, v3 184.7us):
  - PE warmup matmuls during initial h DMA (HAM stays at K=8/8 into proj).
  - kT2 guard columns: 16 tiny self-copies emitted after the last kT2 bias
    add create a real dependency that keeps the Tile scheduler from pulling
    attention scores ahead of the projections (v3's 6us starve + HAM-cold
    restart came from exactly that).
  - V tiles built by HWDGE DMA-transpose instead of PE transposes.
  - Wide 2-bank PSUM score tiles: one exp instruction per (head, 2 kc)
    [128,1024] -> halves the PE<->evac semaphore handshakes that set the
    v2/v3 steady-state cadence (873ns/kc). Head A always Schraudolph on
    DVE (double-buffered), head B always exact exp on ACT (single-buffered).
  - AV matmuls trail the score stream by AV_LAG kc (no deferred block drain,
    no 8us tail); po writeback copies are high-priority so the bufs=1 po
    banks recycle fast.
"""

import sys

for p in ("/opt/pypackages", "/opt/trn_rl_repo"):
    if p not in sys.path:
        sys.path.insert(0, p)

from contextlib import ExitStack

import numpy as np

import concourse.bass as bass
import concourse.mybir as mybir
import concourse.tile as tile
from concourse import bacc
from concourse.bass_utils import run_bass_kernel_spmd

F32 = mybir.dt.float32
BF16 = mybir.dt.bfloat16
I16 = mybir.dt.int16

D_MODEL = 1024
SEQ = 2048
DH = 64
QDIM = 4 * DH       # 256 (4 heads per core)
BS = 2
NG = 4
ND = D_MODEL // 128  # 8 d-chunks
NS = SEQ // 128      # 16 k-chunks
NQS = SEQ // 512     # 4 q-slices

# Schraudolph exp constants: bf16 bits ~= s*128*log2(e) + (127*128 + C)
SCH_MUL = 184.6650390625
SCH_ADD = 16250.5

AV_LAG = 4  # kc-slots the AV matmuls trail the score matmuls


def build_program(zero_bias=False):
    nc = bacc.Bacc("TRN2", target_bir_lowering=False, debug=False)

    hT_d = nc.dram_tensor("hT", [D_MODEL, SEQ], BF16, kind="ExternalInput").ap()
    wqT_d = nc.dram_tensor("wqT", [128, ND * QDIM], BF16, kind="ExternalInput").ap()
    wkvT_d = nc.dram_tensor("wkvT", [128, ND * 128], BF16, kind="ExternalInput").ap()
    bq_d = nc.dram_tensor("bq", [QDIM, 1], F32, kind="ExternalInput").ap()
    bkv_d = nc.dram_tensor("bkv", [128, 1], F32, kind="ExternalInput").ap()
    # out: per head h (4), rows 0-63 = unnormalized O^T features, row 64 = denom
    out_d = nc.dram_tensor("out", [4, DH + 1, SEQ], BF16, kind="ExternalOutput").ap()

    with tile.TileContext(nc) as tc, ExitStack() as ctx:
        sb = ctx.enter_context(tc.tile_pool(name="sb", bufs=1))

        hT = [sb.tile([128, SEQ], BF16, tag=f"hT{d}", name=f"hT{d}") for d in range(ND)]
        wqTa = sb.tile([128, ND * QDIM], BF16, tag="wqTa", name="wqTa")
        wkvTa = sb.tile([128, ND * 128], BF16, tag="wkvTa", name="wkvTa")
        bq0 = sb.tile([128, 1], F32, tag="bq0", name="bq0")
        bq1 = sb.tile([128, 1], F32, tag="bq1", name="bq1")
        bkv = sb.tile([128, 1], F32, tag="bkv", name="bkv")
        qt = [sb.tile([128, SEQ], BF16, tag=f"qt{i}", name=f"qt{i}") for i in range(2)]
        kT2 = sb.tile([128, SEQ], BF16, tag="kT2", name="kT2")
        vT = sb.tile([DH, SEQ], BF16, tag="vT", name="vT")
        vv = [sb.tile([128, DH + 1], BF16, tag=f"vv{i}", name=f"vv{i}") for i in range(NS)]
        wup = sb.tile([128, 128], BF16, tag="wup", name="wup")

        # ---- input DMAs ----
        # h chunks split across the two HWDGE queues (sync+scalar) in d-order;
        # weights/biases via SWDGE (gpsimd) so they don't delay h.
        nc.vector.memset(wup[:, :], 0.0)
        nc.gpsimd.dma_start(bq0[:, :], bq_d[0:128, :])
        nc.gpsimd.dma_start(bq1[:, :], bq_d[128:256, :])
        nc.gpsimd.dma_start(bkv[:, :], bkv_d[:, :])
        # DMA queues are ISSUE-RATE limited (~1.3 transfers/us each), so use
        # few, large transfers: one merged weight DMA per HWDGE queue, the h
        # first halves (all phase 0 needs) interleaved behind them, and the
        # h second halves on the SWDGE queue in parallel (phase 1 chases
        # them d-by-d during phase-0 compute).
        HW = ND * QDIM // 2
        HK = ND * 128 // 2
        nc.sync.dma_start(wqTa[:, 0:HW], wqT_d[:, 0:HW])
        nc.scalar.dma_start(wkvTa[:, 0:HK], wkvT_d[:, 0:HK])
        nc.sync.dma_start(hT[0][:, 0:1024], hT_d[0:128, 0:1024])
        nc.scalar.dma_start(hT[1][:, 0:1024], hT_d[128:256, 0:1024])
        nc.sync.dma_start(wqTa[:, HW:], wqT_d[:, HW:])
        nc.scalar.dma_start(wkvTa[:, HK:], wkvT_d[:, HK:])
        for d in range(2, ND):
            q = nc.sync if d % 2 == 0 else nc.scalar
            q.dma_start(hT[d][:, 0:1024], hT_d[d * 128:(d + 1) * 128, 0:1024])
        for d in range(ND):
            nc.gpsimd.dma_start(hT[d][:, 1024:2048],
                                hT_d[d * 128:(d + 1) * 128, 1024:2048])
        for i in range(NS):
            nc.vector.memset(vv[i][:, DH:DH + 1], 1.0)

        # ---- PE warmup while the first h chunks stream in: keeps the HAM
        # activity window busy so projections start at K=8/8.
        with tc.tile_pool(name="pwu", bufs=2, space="PSUM") as pwu:
            for i in range(28):
                p = pwu.tile([128, 128], F32, tag="pw", name="pw")
                nc.tensor.matmul(p[:, :], wup[:, :], wup[:, :],
                                 start=True, stop=True)

        # ---- projections: 3 matmul targets (q01, q23, kv) per 512-seq slice.
        # d-outer, two slices per phase: 6 PSUM accumulators live; each h
        # chunk is consumed by 6 matmuls the moment its DMA completes.
        with tc.tile_pool(name="pp", bufs=1, space="PSUM") as pp:
            CP = mybir.ActivationFunctionType.Copy

            def evict(s, pt, which):
                # evict one projection target's psum for 512-seq slice s.
                # which: 0 -> q01 heads, 1 -> q23 heads, 2 -> kv
                n0 = s * 512
                if which == 2:
                    if zero_bias:
                        nc.vector.tensor_copy(kT2[0:DH, n0:n0 + 512], pt[0:DH, :])
                        nc.scalar.activation(kT2[DH:128, n0:n0 + 512], pt[0:DH, :], CP)
                        nc.vector.tensor_copy(vT[:, n0:n0 + 512], pt[DH:128, :])
                    else:
                        nc.vector.tensor_scalar_add(kT2[0:DH, n0:n0 + 512], pt[0:DH, :], bkv[0:DH, :])
                        nc.vector.tensor_scalar_add(kT2[DH:128, n0:n0 + 512], pt[0:DH, :], bkv[0:DH, :])
                        nc.vector.tensor_scalar_add(vT[:, n0:n0 + 512], pt[DH:128, :], bkv[DH:128, :])
                    # this vT 128-chunk pair is final: transpose to vv now.
                    # all transposes on ONE queue: the DMA-transpose XBAR is
                    # shared; concurrent transposes from both HWDGE queues
                    # corrupted vv nondeterministically.
                    for j in range(4):
                        i = 4 * s + j
                        nc.sync.dma_start(vv[i][:, 0:DH], vT[:, i * 128:(i + 1) * 128],
                                          transpose=True)
                elif which == 0:
                    if zero_bias:
                        nc.scalar.activation(qt[0][:, n0:n0 + 512], pt[:, :], CP)
                    else:
                        nc.vector.tensor_scalar_add(qt[0][:, n0:n0 + 512], pt[:, :], bq0[:, :])
                else:
                    if zero_bias:
                        nc.vector.tensor_copy(qt[1][:, n0:n0 + 512], pt[:, :])
                    else:
                        nc.vector.tensor_scalar_add(qt[1][:, n0:n0 + 512], pt[:, :], bq1[:, :])

            wsel = [lambda d: wqTa[:, d * QDIM:d * QDIM + 128],
                    lambda d: wqTa[:, d * QDIM + 128:d * QDIM + 256],
                    lambda d: wkvTa[:, d * 128:(d + 1) * 128]]

            # phase 0 (slices 0,1): d-major so each h chunk is consumed the
            # moment its DMA lands; 6 accumulators live.
            acc = {}
            for s in range(2):
                acc[(0, s)] = pp.tile([128, 512], F32, tag=f"p0{s}", name="p0")
                acc[(1, s)] = pp.tile([128, 512], F32, tag=f"p1{s}", name="p1")
                acc[(2, s)] = pp.tile([128, 512], F32, tag=f"pkv{s}", name="pkv")
            for d in range(ND):
                st = dict(start=(d == 0), stop=(d == ND - 1))
                for s in range(2):
                    rhs = hT[d][:, s * 512:s * 512 + 512]
                    for w in (2, 0, 1):
                        nc.tensor.matmul(acc[(w, s)][:, :], wsel[w](d), rhs, **st)
            # kv evictions first: they gate kT2/vv and so the whole attention
            for w in (2, 0, 1):
                for s in range(2):
                    evict(s, acc[(w, s)], w)

            # phase 1 (slices 2,3): d-major again, chasing the second-half
            # h DMAs; kv evictions first so kT2/vv complete earliest.
            acc = {}
            for s in (2, 3):
                acc[(0, s)] = pp.tile([128, 512], F32, tag=f"p0{s % 2}", name="p0")
                acc[(1, s)] = pp.tile([128, 512], F32, tag=f"p1{s % 2}", name="p1")
                acc[(2, s)] = pp.tile([128, 512], F32, tag=f"pkv{s % 2}", name="pkv")
            for d in range(ND):
                st = dict(start=(d == 0), stop=(d == ND - 1))
                for s in (2, 3):
                    rhs = hT[d][:, s * 512:s * 512 + 512]
                    for w in (2, 0, 1):
                        nc.tensor.matmul(acc[(w, s)][:, :], wsel[w](d), rhs, **st)
            for w in (2, 0, 1):
                for s in (2, 3):
                    evict(s, acc[(w, s)], w)

        # Guard: gx = kT2_lastcol*0 + 1 depends on the LAST kv projection
        # output; multiplying one column of each kT2 k-chunk by gx (exact
        # 1.0) makes scores for kc>=2 depend on the full kT2, so the
        # scheduler cannot run attention arbitrarily ahead of the
        # projections (v3 lesson). kc 0-1 stay ungated: they only need
        # kT2 slice 0 (final after phase 0), letting ~16 score pairs and
        # their evacuations pre-fill the pipeline during phase 1.
        gx = sb.tile([128, 1], BF16, tag="gx", name="gx")
        nc.vector.tensor_scalar(gx[:, :], kT2[:, SEQ - 1:SEQ], 0.0, 1.0,
                                op0=mybir.AluOpType.mult, op1=mybir.AluOpType.add)
        for kc in range(2, NS):
            c = kc * 128
            nc.vector.tensor_tensor(kT2[:, c:c + 1], kT2[:, c:c + 1],
                                    gx[:, :], op=mybir.AluOpType.mult)

        # ---- attention ----
        # Flat stream over (hp, qs, kc): score pair into wide 2-bank psum
        # tiles [128, 2*512] covering kc-pairs; one exp instr per (head,
        # kc-pair); AV trails by AV_LAG kc; writeback when a block ends.
        with tc.tile_pool(name="po", bufs=1, space="PSUM") as pop, \
             tc.tile_pool(name="psc", bufs=2, space="PSUM") as psc, \
             tc.tile_pool(name="psw", bufs=2, space="PSUM") as psw, \
             tc.tile_pool(name="ot", bufs=4) as otp, \
             tc.tile_pool(name="at", bufs=AV_LAG + 4) as atp:
            blocks = [(hp, qs) for hp in range(2) for qs in range(NQS)]
            NB = len(blocks)

            po_of = {}    # block index -> (poA, poB)
            at_of = {}    # (bi, kc) -> (atA slice, atB)
            wideA = {}    # bi -> (psA2, at2A) for the current kc-pair

            def scores_step(bi, kc):
                hp, qs = blocks[bi]
                q = qt[hp]
                n0 = qs * 512
                k0 = kc * 128
                # head A: wide 2-bank psum covering a kc-pair; ONE Schraudolph
                # DVE instruction per pair halves the per-kc handshake cost.
                if kc % 2 == 0:
                    psA2 = psw.tile([128, 1024], F32, tag="psA2", name="psA2")
                    at2A = atp.tile([128, 1024], BF16, tag="atA", name="atA")
                    wideA[bi] = (psA2, at2A)
                psA2, at2A = wideA[bi]
                c0 = (kc % 2) * 512
                psB = psc.tile([128, 512], F32, tag="psB", name="psB")
                # (64,128)-tile pair on disjoint contraction-row halves:
                # head A (rows 0-63) and head B (rows 64-127, K duplicate)
                # run concurrently on the PE.
                nc.tensor.matmul(psA2[:, c0:c0 + 512], kT2[0:DH, k0:k0 + 128],
                                 q[0:DH, n0:n0 + 512], start=True, stop=True)
                nc.tensor.matmul(psB[:, :], kT2[DH:128, k0:k0 + 128],
                                 q[DH:128, n0:n0 + 512], start=True, stop=True)
                atB = atp.tile([128, 512], BF16, tag="at", name="at")
                nc.scalar.activation(atB[:, :], psB[:, :],
                                     mybir.ActivationFunctionType.Exp)
                if kc % 2 == 1:
                    nc.vector.tensor_scalar(
                        at2A[:, :].bitcast(I16), psA2[:, :],
                        SCH_MUL, SCH_ADD,
                        op0=mybir.AluOpType.mult, op1=mybir.AluOpType.add)
                at_of[(bi, kc)] = (at2A[:, c0:c0 + 512], atB)

            def av_step(bi, kc):
                if bi not in po_of:
                    po_of[bi] = (pop.tile([DH + 1, 512], F32, tag="poA", name="poA"),
                                 pop.tile([DH + 1, 512], F32, tag="poB", name="poB"))
                poA, poB = po_of[bi]
                atA, atB = at_of.pop((bi, kc))
                st = dict(start=(kc == 0), stop=(kc == NS - 1))
                nc.tensor.matmul(poA[:, :], vv[kc][:, :], atA, **st)
                nc.tensor.matmul(poB[:, :], vv[kc][:, :], atB[:, :], **st)
                if kc == NS - 1:
                    hp, qs = blocks[bi]
                    pn0 = qs * 512
                    otA = otp.tile([DH + 1, 512], BF16, tag="ot", name="ot")
                    otB = otp.tile([DH + 1, 512], BF16, tag="ot", name="ot")
                    # high priority: po has bufs=1, the next block's AV chain
                    # waits on these evictions.
                    with tc.high_priority():
                        nc.scalar.copy(otA[:, :], poA[:, :])
                        nc.vector.tensor_copy(otB[:, :], poB[:, :])
                    nc.sync.dma_start(out_d[2 * hp, :, pn0:pn0 + 512], otA[:, :])
                    nc.gpsimd.dma_start(out_d[2 * hp + 1, :, pn0:pn0 + 512], otB[:, :])
                    del po_of[bi]

            total = NB * NS
            for i in range(total + AV_LAG):
                if i < total:
                    scores_step(i // NS, i % NS)
                else:
                    # drain phase: keep the PE busy so HAM stays warm while
                    # the last exps catch up; dummies rotate through psc
                    # slots (no scores follow, so the garbage is never read)
                    for r in range(2):
                        p = psc.tile([128, 512], F32, tag="psB", name="psB")
                        nc.tensor.matmul(p[0:128, 0:128], wup[:, :], wup[:, :],
                                         start=True, stop=True)
                j = i - AV_LAG
                if j >= 0:
                    av_step(j // NS, j % NS)

    nc.compile()
    return nc


_NC = None
LAST_RESULTS = None
LAST_IN_MAPS = None


def kernel(h, wq_w, wq_b, wk_w, wk_b, wv_w, wv_b, **kw):
    global _NC, LAST_RESULTS, LAST_IN_MAPS
    zb = (np.all(np.asarray(wq_b) == 0) and np.all(np.asarray(wk_b) == 0)
          and np.all(np.asarray(wv_b) == 0))
    if _NC is None:
        _NC = build_program(zero_bias=bool(zb))

    import ml_dtypes
    bf16 = ml_dtypes.bfloat16

    h = np.asarray(h, np.float32)
    wq_w = np.asarray(wq_w, np.float32)
    wq_b = np.asarray(wq_b, np.float32)
    wk_w = np.asarray(wk_w, np.float32)
    wk_b = np.asarray(wk_b, np.float32)
    wv_w = np.asarray(wv_w, np.float32)
    wv_b = np.asarray(wv_b, np.float32)

    in_maps = []
    for core in range(8):
        b, g = divmod(core, NG)
        # fold the 1/sqrt(dh) score scale into wq/bq
        wq_s = wq_w[g * QDIM:(g + 1) * QDIM, :] * 0.125
        bq_s = wq_b[g * QDIM:(g + 1) * QDIM] * 0.125
        wkT = wk_w[g * DH:(g + 1) * DH, :].T            # [1024, 64]
        wvT = wv_w[g * DH:(g + 1) * DH, :].T
        bkv = np.concatenate([wk_b[g * DH:(g + 1) * DH],
                              wv_b[g * DH:(g + 1) * DH]])
        wqT_full = wq_s.T                                  # [1024, 256]
        wkvT_full = np.concatenate([wkT, wvT], axis=1)     # [1024, 128]
        # merged layouts: [128, ND*cols] with d-chunk d at cols [d*cols:...]
        wqTa = np.concatenate([wqT_full[d * 128:(d + 1) * 128, :]
                               for d in range(ND)], axis=1)
        wkvTa = np.concatenate([wkvT_full[d * 128:(d + 1) * 128, :]
                                for d in range(ND)], axis=1)
        in_maps.append({
            "hT": np.ascontiguousarray(h[b].T.astype(bf16)),
            "wqT": np.ascontiguousarray(wqTa.astype(bf16)),
            "wkvT": np.ascontiguousarray(wkvTa.astype(bf16)),
            "bq": np.ascontiguousarray(bq_s.reshape(QDIM, 1)),
            "bkv": np.ascontiguousarray(bkv.reshape(128, 1)),
        })

    res = run_bass_kernel_spmd(_NC, in_maps, core_ids=list(range(8)))
    LAST_RESULTS = res
    LAST_IN_MAPS = in_maps

    out = np.empty((BS, SEQ, 1024), np.float32)
    for core in range(8):
        b, g = divmod(core, NG)
        o = np.asarray(res.results[core]["out"], np.float32)  # [4, 65, 2048]
        on = o[:, 0:DH, :] / o[:, DH:DH + 1, :]  # divide by denominators
        # [4, 64, 2048] -> [2048, 4*64]
        out[b, :, g * QDIM:(g + 1) * QDIM] = (
            on.transpose(2, 0, 1).reshape(SEQ, QDIM))
    return out


# revision 39
# speedup vs baseline: 1.0243x; 1.0243x over previous
"""GQA attention kernel for 8 TRN2 NeuronCores (Bass/Tile) — v4.

Problem: h[2,2048,1024] -> out[2,2048,1024]
  q = h @ wq_w.T + wq_b   (16 heads x 64)
  k/v = h @ w{k,v}_w.T + b (4 KV groups x 64, each serves 4 consecutive heads)
  out = softmax(q k^T / 8) v

Sharding: 8 cores = 2 batches x 4 KV groups (4 query heads each, one shared
K/V group per core). Fully independent, no collectives.

v4 structure (baselines: v2 172.4us, v3 184.7us):
  - PE warmup matmuls during initial h DMA (HAM stays at K=8/8 into proj).
  - kT2 guard columns: 16 tiny self-copies emitted after the last kT2 bias
    add create a real dependency that keeps the Tile scheduler from pulling
    attention scores ahead of the projections (v3's 6us starve + HAM-cold
    restart came from exactly that).
  - V tiles built by HWDGE DMA-transpose instead of PE transposes.
  - Wide 2-bank PSUM score tiles: one exp instruction per (head, 2 kc)
    [128,1024] -> halves the PE<->evac semaphore handshakes that set the
    v2/v3 steady-state cadence (873ns/kc). Head A always Schraudolph on
    DVE (double-buffered), head B always exact exp on ACT (single-buffered).
  - AV matmuls trail the score stream by AV_LAG kc (no deferred block drain,
    no 8us tail); po writeback copies are high-priority so the bufs=1 po
    banks recycle fast.
"""

import sys

for p in ("/opt/pypackages", "/opt/trn_rl_repo"):
    if p not in sys.path:
        sys.path.insert(0, p)

from contextlib import ExitStack

import numpy as np

import concourse.bass as bass
import concourse.mybir as mybir
import concourse.tile as tile
from concourse import bacc
from concourse.bass_utils import run_bass_kernel_spmd

F32 = mybir.dt.float32
BF16 = mybir.dt.bfloat16
I16 = mybir.dt.int16

D_MODEL = 1024
SEQ = 2048
DH = 64
QDIM = 4 * DH       # 256 (4 heads per core)
BS = 2
NG = 4
ND = D_MODEL // 128  # 8 d-chunks
NS = SEQ // 128      # 16 k-chunks
NQS = SEQ // 512     # 4 q-slices

# Schraudolph exp constants: bf16 bits ~= s*128*log2(e) + (127*128 + C)
SCH_MUL = 184.6650390625
SCH_ADD = 16250.5

AV_LAG = 5  # kc-slots the AV matmuls trail the score matmuls


def build_program(zero_bias=False):
    nc = bacc.Bacc("TRN2", target_bir_lowering=False, debug=False)

    hT_d = nc.dram_tensor("hT", [D_MODEL, SEQ], BF16, kind="ExternalInput").ap()
    wqT_d = nc.dram_tensor("wqT", [128, ND * QDIM], BF16, kind="ExternalInput").ap()
    wkvT_d = nc.dram_tensor("wkvT", [128, ND * 128], BF16, kind="ExternalInput").ap()
    bq_d = nc.dram_tensor("bq", [QDIM, 1], F32, kind="ExternalInput").ap()
    bkv_d = nc.dram_tensor("bkv", [128, 1], F32, kind="ExternalInput").ap()
    # out: per head h (4), rows 0-63 = unnormalized O^T features, row 64 = denom
    out_d = nc.dram_tensor("out", [4, DH + 1, SEQ], BF16, kind="ExternalOutput").ap()

    with tile.TileContext(nc) as tc, ExitStack() as ctx:
        sb = ctx.enter_context(tc.tile_pool(name="sb", bufs=1))

        hT = [sb.tile([128, SEQ], BF16, tag=f"hT{d}", name=f"hT{d}") for d in range(ND)]
        wqTa = sb.tile([128, ND * QDIM], BF16, tag="wqTa", name="wqTa")
        wkvTa = sb.tile([128, ND * 128], BF16, tag="wkvTa", name="wkvTa")
        bq0 = sb.tile([128, 1], F32, tag="bq0", name="bq0")
        bq1 = sb.tile([128, 1], F32, tag="bq1", name="bq1")
        bkv = sb.tile([128, 1], F32, tag="bkv", name="bkv")
        qt = [sb.tile([128, SEQ], BF16, tag=f"qt{i}", name=f"qt{i}") for i in range(2)]
        kT2 = sb.tile([128, SEQ], BF16, tag="kT2", name="kT2")
        vT = sb.tile([DH, SEQ], BF16, tag="vT", name="vT")
        vv = [sb.tile([128, DH + 1], BF16, tag=f"vv{i}", name=f"vv{i}") for i in range(NS)]
        wup = sb.tile([128, 128], BF16, tag="wup", name="wup")

        # ---- input DMAs ----
        # h chunks split across the two HWDGE queues (sync+scalar) in d-order;
        # weights/biases via SWDGE (gpsimd) so they don't delay h.
        nc.vector.memset(wup[:, :], 0.0)
        nc.gpsimd.dma_start(bq0[:, :], bq_d[0:128, :])
        nc.gpsimd.dma_start(bq1[:, :], bq_d[128:256, :])
        nc.gpsimd.dma_start(bkv[:, :], bkv_d[:, :])
        # DMA queues are ISSUE-RATE limited (~1.3 transfers/us each), so use
        # few, large transfers: one merged weight DMA per HWDGE queue, the h
        # first halves (all phase 0 needs) interleaved behind them, and the
        # h second halves on the SWDGE queue in parallel (phase 1 chases
        # them d-by-d during phase-0 compute).
        HW = ND * QDIM // 2
        HK = ND * 128 // 2
        nc.sync.dma_start(wqTa[:, 0:HW], wqT_d[:, 0:HW])
        nc.scalar.dma_start(wkvTa[:, 0:HK], wkvT_d[:, 0:HK])
        nc.sync.dma_start(hT[0][:, 0:1024], hT_d[0:128, 0:1024])
        nc.scalar.dma_start(hT[1][:, 0:1024], hT_d[128:256, 0:1024])
        nc.sync.dma_start(wqTa[:, HW:], wqT_d[:, HW:])
        nc.scalar.dma_start(wkvTa[:, HK:], wkvT_d[:, HK:])
        for d in range(2, ND):
            q = nc.sync if d % 2 == 0 else nc.scalar
            q.dma_start(hT[d][:, 0:1024], hT_d[d * 128:(d + 1) * 128, 0:1024])
        for d in range(ND):
            nc.gpsimd.dma_start(hT[d][:, 1024:2048],
                                hT_d[d * 128:(d + 1) * 128, 1024:2048])
        for i in range(NS):
            nc.vector.memset(vv[i][:, DH:DH + 1], 1.0)

        # ---- PE warmup while the first h chunks stream in: keeps the HAM
        # activity window busy so projections start at K=8/8.
        with tc.tile_pool(name="pwu", bufs=2, space="PSUM") as pwu:
            for i in range(28):
                p = pwu.tile([128, 128], F32, tag="pw", name="pw")
                nc.tensor.matmul(p[:, :], wup[:, :], wup[:, :],
                                 start=True, stop=True)

        # ---- projections: 3 matmul targets (q01, q23, kv) per 512-seq slice.
        # d-outer, two slices per phase: 6 PSUM accumulators live; each h
        # chunk is consumed by 6 matmuls the moment its DMA completes.
        with tc.tile_pool(name="pp", bufs=1, space="PSUM") as pp:
            CP = mybir.ActivationFunctionType.Copy

            def evict(s, pt, which):
                # evict one projection target's psum for 512-seq slice s.
                # which: 0 -> q01 heads, 1 -> q23 heads, 2 -> kv
                n0 = s * 512
                if which == 2:
                    if zero_bias:
                        nc.vector.tensor_copy(kT2[0:DH, n0:n0 + 512], pt[0:DH, :])
                        nc.scalar.activation(kT2[DH:128, n0:n0 + 512], pt[0:DH, :], CP)
                        nc.vector.tensor_copy(vT[:, n0:n0 + 512], pt[DH:128, :])
                    else:
                        nc.vector.tensor_scalar_add(kT2[0:DH, n0:n0 + 512], pt[0:DH, :], bkv[0:DH, :])
                        nc.vector.tensor_scalar_add(kT2[DH:128, n0:n0 + 512], pt[0:DH, :], bkv[0:DH, :])
                        nc.vector.tensor_scalar_add(vT[:, n0:n0 + 512], pt[DH:128, :], bkv[DH:128, :])
                    # this vT 128-chunk pair is final: transpose to vv now.
                    # all transposes on ONE queue: the DMA-transpose XBAR is
                    # shared; concurrent transposes from both HWDGE queues
                    # corrupted vv nondeterministically.
                    for j in range(4):
                        i = 4 * s + j
                        nc.sync.dma_start(vv[i][:, 0:DH], vT[:, i * 128:(i + 1) * 128],
                                          transpose=True)
                elif which == 0:
                    if zero_bias:
                        nc.scalar.activation(qt[0][:, n0:n0 + 512], pt[:, :], CP)
                    else:
                        nc.vector.tensor_scalar_add(qt[0][:, n0:n0 + 512], pt[:, :], bq0[:, :])
                else:
                    if zero_bias:
                        nc.vector.tensor_copy(qt[1][:, n0:n0 + 512], pt[:, :])
                    else:
                        nc.vector.tensor_scalar_add(qt[1][:, n0:n0 + 512], pt[:, :], bq1[:, :])

            wsel = [lambda d: wqTa[:, d * QDIM:d * QDIM + 128],
                    lambda d: wqTa[:, d * QDIM + 128:d * QDIM + 256],
                    lambda d: wkvTa[:, d * 128:(d + 1) * 128]]

            # phase 0 (slices 0,1): d-major so each h chunk is consumed the
            # moment its DMA lands; 6 accumulators live.
            acc = {}
            for s in range(2):
                acc[(0, s)] = pp.tile([128, 512], F32, tag=f"p0{s}", name="p0")
                acc[(1, s)] = pp.tile([128, 512], F32, tag=f"p1{s}", name="p1")
                acc[(2, s)] = pp.tile([128, 512], F32, tag=f"pkv{s}", name="pkv")
            for d in range(ND):
                st = dict(start=(d == 0), stop=(d == ND - 1))
                for s in range(2):
                    rhs = hT[d][:, s * 512:s * 512 + 512]
                    for w in (2, 0, 1):
                        nc.tensor.matmul(acc[(w, s)][:, :], wsel[w](d), rhs, **st)
            # kv evictions first: they gate kT2/vv and so the whole attention
            for w in (2, 0, 1):
                for s in range(2):
                    evict(s, acc[(w, s)], w)

            # phase 1 (slices 2,3): d-major again, chasing the second-half
            # h DMAs; kv evictions first so kT2/vv complete earliest.
            acc = {}
            for s in (2, 3):
                acc[(0, s)] = pp.tile([128, 512], F32, tag=f"p0{s % 2}", name="p0")
                acc[(1, s)] = pp.tile([128, 512], F32, tag=f"p1{s % 2}", name="p1")
                acc[(2, s)] = pp.tile([128, 512], F32, tag=f"pkv{s % 2}", name="pkv")
            for d in range(ND):
                st = dict(start=(d == 0), stop=(d == ND - 1))
                for s in (2, 3):
                    rhs = hT[d][:, s * 512:s * 512 + 512]
                    for w in (2, 0, 1):
                        nc.tensor.matmul(acc[(w, s)][:, :], wsel[w](d), rhs, **st)
            for w in (2, 0, 1):
                for s in (2, 3):
                    evict(s, acc[(w, s)], w)

        # Guard: gx = kT2_lastcol*0 + 1 depends on the LAST kv projection
        # output; multiplying one column of each kT2 k-chunk by gx (exact
        # 1.0) makes scores for kc>=2 depend on the full kT2, so the
        # scheduler cannot run attention arbitrarily ahead of the
        # projections (v3 lesson). kc 0-1 stay ungated: they only need
        # kT2 slice 0 (final after phase 0), letting ~16 score pairs and
        # their evacuations pre-fill the pipeline during phase 1.
        gx = sb.tile([128, 1], BF16, tag="gx", name="gx")
        nc.vector.tensor_scalar(gx[:, :], kT2[:, SEQ - 1:SEQ], 0.0, 1.0,
                                op0=mybir.AluOpType.mult, op1=mybir.AluOpType.add)
        for kc in range(2, NS):
            c = kc * 128
            nc.vector.tensor_tensor(kT2[:, c:c + 1], kT2[:, c:c + 1],
                                    gx[:, :], op=mybir.AluOpType.mult)

        # ---- attention ----
        # Flat stream over (hp, qs, kc): score pair into wide 2-bank psum
        # tiles [128, 2*512] covering kc-pairs; one exp instr per (head,
        # kc-pair); AV trails by AV_LAG kc; writeback when a block ends.
        with tc.tile_pool(name="po", bufs=1, space="PSUM") as pop, \
             tc.tile_pool(name="psc", bufs=2, space="PSUM") as psc, \
             tc.tile_pool(name="psw", bufs=2, space="PSUM") as psw, \
             tc.tile_pool(name="ot", bufs=4) as otp, \
             tc.tile_pool(name="at", bufs=AV_LAG + 4) as atp:
            blocks = [(hp, qs) for hp in range(2) for qs in range(NQS)]
            NB = len(blocks)

            po_of = {}    # block index -> (poA, poB)
            at_of = {}    # (bi, kc) -> (atA slice, atB)
            wideA = {}    # bi -> (psA2, at2A) for the current kc-pair

            def scores_step(bi, kc):
                hp, qs = blocks[bi]
                q = qt[hp]
                n0 = qs * 512
                k0 = kc * 128
                # head A: wide 2-bank psum covering a kc-pair; ONE Schraudolph
                # DVE instruction per pair halves the per-kc handshake cost.
                if kc % 2 == 0:
                    psA2 = psw.tile([128, 1024], F32, tag="psA2", name="psA2")
                    at2A = atp.tile([128, 1024], BF16, tag="atA", name="atA")
                    wideA[bi] = (psA2, at2A)
                psA2, at2A = wideA[bi]
                c0 = (kc % 2) * 512
                psB = psc.tile([128, 512], F32, tag="psB", name="psB")
                # (64,128)-tile pair on disjoint contraction-row halves:
                # head A (rows 0-63) and head B (rows 64-127, K duplicate)
                # run concurrently on the PE.
                nc.tensor.matmul(psA2[:, c0:c0 + 512], kT2[0:DH, k0:k0 + 128],
                                 q[0:DH, n0:n0 + 512], start=True, stop=True)
                nc.tensor.matmul(psB[:, :], kT2[DH:128, k0:k0 + 128],
                                 q[DH:128, n0:n0 + 512], start=True, stop=True)
                atB = atp.tile([128, 512], BF16, tag="at", name="at")
                nc.scalar.activation(atB[:, :], psB[:, :],
                                     mybir.ActivationFunctionType.Exp)
                if kc % 2 == 1:
                    nc.vector.tensor_scalar(
                        at2A[:, :].bitcast(I16), psA2[:, :],
                        SCH_MUL, SCH_ADD,
                        op0=mybir.AluOpType.mult, op1=mybir.AluOpType.add)
                at_of[(bi, kc)] = (at2A[:, c0:c0 + 512], atB)

            def av_step(bi, kc):
                if bi not in po_of:
                    po_of[bi] = (pop.tile([DH + 1, 512], F32, tag="poA", name="poA"),
                                 pop.tile([DH + 1, 512], F32, tag="poB", name="poB"))
                poA, poB = po_of[bi]
                atA, atB = at_of.pop((bi, kc))
                st = dict(start=(kc == 0), stop=(kc == NS - 1))
                nc.tensor.matmul(poA[:, :], vv[kc][:, :], atA, **st)
                nc.tensor.matmul(poB[:, :], vv[kc][:, :], atB[:, :], **st)
                if kc == NS - 1:
                    hp, qs = blocks[bi]
                    pn0 = qs * 512
                    otA = otp.tile([DH + 1, 512], BF16, tag="ot", name="ot")
                    otB = otp.tile([DH + 1, 512], BF16, tag="ot", name="ot")
                    # high priority: po has bufs=1, the next block's AV chain
                    # waits on these evictions.
                    with tc.high_priority():
                        nc.scalar.copy(otA[:, :], poA[:, :])
                        nc.vector.tensor_copy(otB[:, :], poB[:, :])
                    nc.sync.dma_start(out_d[2 * hp, :, pn0:pn0 + 512], otA[:, :])
                    nc.gpsimd.dma_start(out_d[2 * hp + 1, :, pn0:pn0 + 512], otB[:, :])
                    del po_of[bi]

            total = NB * NS
            for i in range(total + AV_LAG):
                if i < total:
                    scores_step(i // NS, i % NS)
                else:
                    # drain phase: keep the PE busy so HAM stays warm while
                    # the last exps catch up; dummies rotate through psc
                    # slots (no scores follow, so the garbage is never read)
                    for r in range(2):
                        p = psc.tile([128, 512], F32, tag="psB", name="psB")
                        nc.tensor.matmul(p[0:128, 0:128], wup[:, :], wup[:, :],
                                         start=True, stop=True)
                j = i - AV_LAG
                if j >= 0:
                    av_step(j // NS, j % NS)

    nc.compile()
    return nc


_NC = None
LAST_RESULTS = None
LAST_IN_MAPS = None


def kernel(h, wq_w, wq_b, wk_w, wk_b, wv_w, wv_b, **kw):
    global _NC, LAST_RESULTS, LAST_IN_MAPS
    zb = (np.all(np.asarray(wq_b) == 0) and np.all(np.asarray(wk_b) == 0)
          and np.all(np.asarray(wv_b) == 0))
    if _NC is None:
        _NC = build_program(zero_bias=bool(zb))

    import ml_dtypes
    bf16 = ml_dtypes.bfloat16

    h = np.asarray(h, np.float32)
    wq_w = np.asarray(wq_w, np.float32)
    wq_b = np.asarray(wq_b, np.float32)
    wk_w = np.asarray(wk_w, np.float32)
    wk_b = np.asarray(wk_b, np.float32)
    wv_w = np.asarray(wv_w, np.float32)
    wv_b = np.asarray(wv_b, np.float32)

    in_maps = []
    for core in range(8):
        b, g = divmod(core, NG)
        # fold the 1/sqrt(dh) score scale into wq/bq
        wq_s = wq_w[g * QDIM:(g + 1) * QDIM, :] * 0.125
        bq_s = wq_b[g * QDIM:(g + 1) * QDIM] * 0.125
        wkT = wk_w[g * DH:(g + 1) * DH, :].T            # [1024, 64]
        wvT = wv_w[g * DH:(g + 1) * DH, :].T
        bkv = np.concatenate([wk_b[g * DH:(g + 1) * DH],
                              wv_b[g * DH:(g + 1) * DH]])
        wqT_full = wq_s.T                                  # [1024, 256]
        wkvT_full = np.concatenate([wkT, wvT], axis=1)     # [1024, 128]
        # merged layouts: [128, ND*cols] with d-chunk d at cols [d*cols:...]
        wqTa = np.concatenate([wqT_full[d * 128:(d + 1) * 128, :]
                               for d in range(ND)], axis=1)
        wkvTa = np.concatenate([wkvT_full[d * 128:(d + 1) * 128, :]
                                for d in range(ND)], axis=1)
        in_maps.append({
            "hT": np.ascontiguousarray(h[b].T.astype(bf16)),
            "wqT": np.ascontiguousarray(wqTa.astype(bf16)),
            "wkvT": np.ascontiguousarray(wkvTa.astype(bf16)),
            "bq": np.ascontiguousarray(bq_s.reshape(QDIM, 1)),
            "bkv": np.ascontiguousarray(bkv.reshape(128, 1)),
        })

    res = run_bass_kernel_spmd(_NC, in_maps, core_ids=list(range(8)))
    LAST_RESULTS = res
    LAST_IN_MAPS = in_maps

    out = np.empty((BS, SEQ, 1024), np.float32)
    for core in range(8):
        b, g = divmod(core, NG)
        o = np.asarray(res.results[core]["out"], np.float32)  # [4, 65, 2048]
        on = o[:, 0:DH, :] / o[:, DH:DH + 1, :]  # divide by denominators
        # [4, 64, 2048] -> [2048, 4*64]
        out[b, :, g * QDIM:(g + 1) * QDIM] = (
            on.transpose(2, 0, 1).reshape(SEQ, QDIM))
    return out
